# revision 10
# baseline (speedup 1.0000x reference)
"""AttentionBlock (GroupNorm + single-head-group attention + out-proj + residual)
for Trainium2, data-parallel over batch across 8 NeuronCores.

Reference computation (per batch element, fp32 reference):
  hn  = GroupNorm32(x)                      # x: (C=512, L=1024)
  q   = Wq @ hn + bq ; kv = Wkv @ hn + bkv ; k, v = split(kv)
  per head h (8 heads, dh=64):
    dots = (q*s)^T (k*s), s = dh^-0.5       # scale applied to both q and k
    attn = softmax(dots, axis=s)
    out  = attn @ v^T  -> (dh, L)
  y = Wo @ out + bo + x

Layout strategy (avoids all large transposes):
  - channels on partitions for x/hn/q/k; v computed TRANSPOSED (L on
    partitions) directly from the projection (lhsT=hn);
  - scores computed transposed: dotsT[s,t] = kh^T qh (lhsT=kh, rhs=qh);
  - head pairs (2h, 2h+1) live at partition bases 0/64 of one chunk, so
    their K=64 dots matmuls run CONCURRENTLY in disjoint PE row groups
    via tile_position=(0,0)/(64,0);
  - softmax denominator via a ones-column appended to the vT weight tile
    (row 64 of the AV psum = sum_s exp); normalization applied at AV
    evacuation with a rank-1 (K=1) broadcast matmul;
  - exp() numerically safe without max-subtraction: |dots| < 0.5 here;
  - matmul operands in bf16 (fp32 matmul costs 2x on the PE and disables
    fast weight load); psum accumulation, groupnorm statistics, softmax
    normalization and the residual add stay fp32.
"""

import numpy as np
import ml_dtypes

import concourse.bass as bass
import concourse.mybir as mybir
import concourse.tile as tile
from concourse import bacc, bass_utils
from concourse.bass import ts

F32 = mybir.dt.float32
BF16 = mybir.dt.bfloat16
AF = mybir.ActivationFunctionType
OP = mybir.AluOpType

B = 8
C = 512
HW = 32
L = HW * HW  # 1024
H = 8
DH = C // H  # 64
G = 32
GS = C // G  # 16
EPS = 1e-5
SCALE2 = float(DH) ** -1.0  # (dh^-0.5) applied to BOTH q and k -> 1/dh on dots
P = 128
CCH = C // P  # 4 channel chunks
LCH = L // P  # 8 L chunks
NCORES = 8
VW = H * (DH + 1)  # 520: v^T tiles hold [64 v cols + 1 ones col] per head


def _body(tc, tensors):
    nc = tc.nc
    from contextlib import ExitStack

    ctx = ExitStack()
    with ctx:
        persist = ctx.enter_context(tc.tile_pool(name="persist", bufs=1))
        work = ctx.enter_context(tc.tile_pool(name="work", bufs=4))
        expp = ctx.enter_context(tc.tile_pool(name="expp", bufs=18))
        outp = ctx.enter_context(tc.tile_pool(name="outp", bufs=3))
        ps_wide = ctx.enter_context(tc.tile_pool(name="ps_wide", bufs=2, space="PSUM"))
        ps_av = ctx.enter_context(tc.tile_pool(name="ps_av", bufs=2, space="PSUM"))
        ps_bc = ctx.enter_context(tc.tile_pool(name="ps_bc", bufs=2, space="PSUM"))
        ps_small = ps_bc

        x_d = tensors["x"].ap()
        gamma_d = tensors["gamma"].ap()
        beta_d = tensors["beta"].ap()
        bq_d = tensors["bq"].ap()
        bkv_d = tensors["bkv"].ap()
        bo_d = tensors["bo"].ap()
        wqT_d = tensors["wqT"].ap()
        wkvT_d = tensors["wkvT"].ap()
        woT_d = tensors["woT"].ap()
        ind_d = tensors["ind"].ap()
        indT_d = tensors["indT"].ap()
        out_d = tensors["out"].ap()

        # ---------------- load inputs ----------------
        xt = []  # x, channel chunks (128, L)
        x3 = x_d.rearrange("(cc p) l -> cc p l", p=P)
        for cj in range(CCH):
            t = persist.tile([P, L], F32, tag=f"x{cj}")
            nc.sync.dma_start(t, x3[cj])
            xt.append(t)

        # per-channel params as (128, CCH) columns; column cj <-> channels cj*128..+128
        def load_cols(dram_vec, name, lo=0, n=C):
            t = persist.tile([P, CCH], F32, tag=name)
            nc.sync.dma_start(t, dram_vec[lo : lo + n].rearrange("(o p) -> p o", p=P))
            return t

        gamma_t = load_cols(gamma_d, "gamma")
        beta_t = load_cols(beta_d, "beta")
        bq_t = load_cols(bq_d, "bq")
        bk_t = load_cols(bkv_d, "bk", 0, C)
        bv_t = load_cols(bkv_d, "bv", C, C)
        bo_t = load_cols(bo_d, "bo")

        wq_t = persist.tile([P, CCH, C], BF16, tag="wq")
        nc.sync.dma_start(wq_t, wqT_d.rearrange("(cc p) o -> p cc o", p=P))
        wkv_t = persist.tile([P, CCH, 2 * C], BF16, tag="wkv")
        nc.sync.dma_start(wkv_t, wkvT_d.rearrange("(cc p) o -> p cc o", p=P))
        wo_t = persist.tile([P, CCH, C], BF16, tag="wo")
        nc.sync.dma_start(wo_t, woT_d.rearrange("(cc p) o -> p cc o", p=P))

        ind_t = persist.tile([P, CCH, G], F32, tag="ind")
        nc.sync.dma_start(ind_t, ind_d.rearrange("(cc p) g -> p cc g", p=P))
        indT_t = persist.tile([G, C], F32, tag="indT")
        nc.sync.dma_start(indT_t, indT_d)

        ones1 = persist.tile([1, DH], F32, tag="ones1")
        nc.vector.memset(ones1, 1.0)
        eps_t = persist.tile([G, 1], F32, tag="eps")
        nc.vector.memset(eps_t, EPS)

        # ---------------- GroupNorm ----------------
        # per-channel [sum, sumsq] -> group-reduce via indicator matmul
        stats = work.tile([P, CCH, 2], F32, tag="stats")
        for cj in range(CCH):
            sq = work.tile([P, L], F32, tag="sq")
            nc.scalar.activation(sq, xt[cj], AF.Square, accum_out=stats[:, cj, 1:2])
            nc.vector.reduce_sum(stats[:, cj, 0:1], xt[cj], axis=mybir.AxisListType.X)

        ps_stats = ps_small.tile([G, 2], F32, tag="ps")
        for cj in range(CCH):
            nc.tensor.matmul(
                ps_stats,
                ind_t[:, cj, :],
                stats[:, cj, :],
                start=(cj == 0),
                stop=(cj == CCH - 1),
            )

        # mv = [mean, rstd] per group (G partitions)
        mv = work.tile([G, 2], F32, tag="mv")
        inv_n = 1.0 / (GS * L)
        nc.scalar.mul(mv[:, 0:1], ps_stats[:, 0:1], inv_n)  # mean
        nc.scalar.mul(mv[:, 1:2], ps_stats[:, 1:2], inv_n)  # E[x^2]
        musq = work.tile([G, 1], F32, tag="musq")
        nc.vector.tensor_mul(musq, mv[:, 0:1], mv[:, 0:1])
        nc.vector.tensor_tensor(mv[:, 1:2], mv[:, 1:2], musq, OP.subtract)  # var
        nc.scalar.activation(mv[:, 1:2], mv[:, 1:2], AF.Sqrt, bias=eps_t)
        nc.vector.reciprocal(mv[:, 1:2], mv[:, 1:2])  # rstd

        # broadcast group stats back to channels: (G,2) -> (128,2) per chunk
        hn = []
        for cj in range(CCH):
            ps_bcst = ps_small.tile([P, 2], F32, tag="ps")
            nc.tensor.matmul(ps_bcst, indT_t[:, ts(cj, P)], mv, start=True, stop=True)
            mc = work.tile([P, 2], F32, tag="mc")
            nc.vector.tensor_copy(mc, ps_bcst)
            a = work.tile([P, 1], F32, tag="a_sc")
            b = work.tile([P, 1], F32, tag="b_sc")
            # a = rstd*gamma ; b = beta - mean*a
            nc.vector.tensor_mul(a, mc[:, 1:2], gamma_t[:, cj : cj + 1])
            nc.vector.tensor_mul(b, mc[:, 0:1], a)
            nc.vector.tensor_tensor(b, beta_t[:, cj : cj + 1], b, OP.subtract)
            t = persist.tile([P, L], BF16, tag=f"hn{cj}")
            nc.vector.tensor_scalar(
                t, xt[cj], scalar1=a, scalar2=b, op0=OP.mult, op1=OP.add
            )
            hn.append(t)

        # ---------------- projections ----------------
        # q (channels on partitions), pre-scaled by 1/dh; k (channels on partitions);
        # vT (L on partitions) with per-head ones-columns appended.
        q_t = [persist.tile([P, L], BF16, tag=f"q{oj}", name=f"q{oj}") for oj in range(CCH)]
        k_t = [persist.tile([P, L], BF16, tag=f"k{oj}", name=f"k{oj}") for oj in range(CCH)]
        for oj in range(CCH):
            for th in range(2):
                ps_q = ps_av.tile([P, 512], F32, tag="ps")
                for cj in range(CCH):
                    nc.tensor.matmul(
                        ps_q,
                        wq_t[:, cj, ts(oj, P)],
                        hn[cj][:, ts(th, 512)],
                        start=(cj == 0),
                        stop=(cj == CCH - 1),
                    )
                # q = (psum + bq) * (1/dh)
                nc.vector.tensor_scalar(
                    q_t[oj][:, ts(th, 512)],
                    ps_q,
                    scalar1=bq_t[:, oj : oj + 1],
                    scalar2=SCALE2,
                    op0=OP.add,
                    op1=OP.mult,
                )
                ps_k = ps_av.tile([P, 512], F32, tag="ps")
                for cj in range(CCH):
                    nc.tensor.matmul(
                        ps_k,
                        wkv_t[:, cj, ts(oj, P)],
                        hn[cj][:, ts(th, 512)],
                        start=(cj == 0),
                        stop=(cj == CCH - 1),
                    )
                nc.vector.tensor_scalar(
                    k_t[oj][:, ts(th, 512)],
                    ps_k,
                    scalar1=bk_t[:, oj : oj + 1],
                    scalar2=None,
                    op0=OP.add,
                )

        # vT: out[l, i] = sum_c hn[c, l] * Wv^T[c, i]  (lhsT = hn chunks)
        vT = [persist.tile([P, VW], BF16, tag=f"vT{lj}", name=f"vT{lj}") for lj in range(LCH)]
        for lj in range(LCH):
            v3 = vT[lj].rearrange("p (h w) -> p h w", w=DH + 1)
            nc.vector.memset(v3[:, :, DH : DH + 1], 1.0)
            ps_v = ps_av.tile([P, 512], F32, tag="ps")
            for cj in range(CCH):
                nc.tensor.matmul(
                    ps_v,
                    hn[cj][:, ts(lj, P)],
                    wkv_t[:, cj, C : 2 * C],
                    start=(cj == 0),
                    stop=(cj == CCH - 1),
                )
            # v bias is folded into the attention output (rows sum to 1).
            # single strided copy: psum (p,(h d)) -> vT (p,h,0:DH)
            nc.vector.tensor_copy(
                v3[:, :, 0:DH], ps_v.rearrange("p (h d) -> p h d", d=DH)
            )

        # ---------------- attention, head-pair pipelined ----------------
        av_t = [persist.tile([P, L], BF16, tag=f"av{oj}", name=f"av{oj}") for oj in range(CCH)]
        exp_tiles: dict = {}

        def emit_dots_exp_pair(hp):
            oj = hp
            qA, qB = q_t[oj][0:DH, :], q_t[oj][DH:P, :]
            kA, kB = k_t[oj][0:DH, :], k_t[oj][DH:P, :]
            for sj in range(LCH):
                psA = ps_wide.tile([P, L], F32, tag="ps")
                psB = ps_wide.tile([P, L], F32, tag="ps")
                for th in range(2):
                    nc.tensor.matmul(
                        psA[:, ts(th, 512)],
                        kA[:, ts(sj, P)],
                        qA[:, ts(th, 512)],
                        start=True,
                        stop=True,
                        tile_position=(0, 0),
                    )
                    nc.tensor.matmul(
                        psB[:, ts(th, 512)],
                        kB[:, ts(sj, P)],
                        qB[:, ts(th, 512)],
                        start=True,
                        stop=True,
                        tile_position=(64, 0),
                    )
                eA = expp.tile([P, L], BF16, tag="exp")
                nc.scalar.activation(eA, psA, AF.Exp)
                eB = expp.tile([P, L], BF16, tag="exp")
                nc.scalar.activation(eB, psB, AF.Exp)
                exp_tiles[(2 * hp, sj)] = eA
                exp_tiles[(2 * hp + 1, sj)] = eB

        def emit_av(h):
            oj, base = h // 2, DH * (h % 2)
            w0 = h * (DH + 1)
            for th in range(2):
                ps_o = ps_av.tile([DH + 1, 512], F32, tag="ps")
                for sj in range(LCH):
                    nc.tensor.matmul(
                        ps_o,
                        vT[sj][:, w0 : w0 + DH + 1],
                        exp_tiles[(h, sj)][:, ts(th, 512)],
                        start=(sj == 0),
                        stop=(sj == LCH - 1),
                    )
                rec = work.tile([1, 512], F32, tag="rec")
                nc.vector.reciprocal(rec, ps_o[DH : DH + 1, :])
                ps_b = ps_bc.tile([DH, 512], F32, tag="ps")
                nc.tensor.matmul(ps_b, ones1, rec, start=True, stop=True)
                bc = work.tile([DH, 512], F32, tag="bc")
                nc.vector.tensor_copy(bc, ps_b)
                nc.vector.tensor_tensor(
                    av_t[oj][base : base + DH, ts(th, 512)],
                    ps_o[:DH, :],
                    bc,
                    OP.mult,
                )
            for sj in range(LCH):
                del exp_tiles[(h, sj)]

        emit_dots_exp_pair(0)
        for hp in range(1, CCH):
            emit_dots_exp_pair(hp)
            emit_av(2 * (hp - 1))
            emit_av(2 * (hp - 1) + 1)
        emit_av(H - 2)
        emit_av(H - 1)

        # v bias: out[d,t] += bv[d]  (rows of av are inner channels)
        # (softmax rows sum to one, so +bv on v adds bv to every column)
        for oj in range(CCH):
            nc.vector.tensor_scalar(
                av_t[oj], av_t[oj], scalar1=bv_t[:, oj : oj + 1], scalar2=None, op0=OP.add
            )

        # ---------------- output projection + residual ----------------
        out3 = out_d.rearrange("(cc p) l -> cc p l", p=P)
        for oj in range(CCH):
            ot = outp.tile([P, L], F32, tag="ot")
            for th in range(2):
                ps_f = ps_av.tile([P, 512], F32, tag="ps")
                for cj in range(CCH):
                    nc.tensor.matmul(
                        ps_f,
                        wo_t[:, cj, ts(oj, P)],
                        av_t[cj][:, ts(th, 512)],
                        start=(cj == 0),
                        stop=(cj == CCH - 1),
                    )
                nc.vector.tensor_scalar(
                    ot[:, ts(th, 512)],
                    ps_f,
                    scalar1=bo_t[:, oj : oj + 1],
                    scalar2=None,
                    op0=OP.add,
                )
            nc.vector.tensor_add(ot, ot, xt[oj])
            nc.sync.dma_start(out3[oj], ot)


_CACHE = {}


def _build():
    if "nc" in _CACHE:
        return _CACHE["nc"]
    nc = bacc.Bacc("TRN2", target_bir_lowering=False, debug=False, num_devices=NCORES)
    tensors = {}
    specs = [
        ("x", (C, L), F32),
        ("gamma", (C,), F32),
        ("beta", (C,), F32),
        ("bq", (C,), F32),
        ("bkv", (2 * C,), F32),
        ("bo", (C,), F32),
        ("wqT", (C, C), BF16),
        ("wkvT", (C, 2 * C), BF16),
        ("woT", (C, C), BF16),
        ("ind", (C, G), F32),
        ("indT", (G, C), F32),
    ]
    for name, shape, dt in specs:
        tensors[name] = nc.dram_tensor(name, shape, dt, kind="ExternalInput")
    tensors["out"] = nc.dram_tensor("out", (C, L), F32, kind="ExternalOutput")
    with tile.TileContext(nc) as tc:
        _body(tc, tensors)
    nc.compile()
    _CACHE["nc"] = nc
    return nc


def _in_maps(x, gamma, beta, Wq, bq, Wkv, bkv, Wo, bo):
    f32 = lambda a: np.ascontiguousarray(np.asarray(a, dtype=np.float32))
    bf16 = lambda a: np.ascontiguousarray(
        np.asarray(a, dtype=np.float32).astype(ml_dtypes.bfloat16)
    )
    xr = f32(x).reshape(B, C, L)
    ind = np.zeros((C, G), np.float32)
    ind[np.arange(C), np.arange(C) // GS] = 1.0
    shared = {
        "gamma": f32(gamma),
        "beta": f32(beta),
        "bq": f32(bq),
        "bkv": f32(bkv),
        "bo": f32(bo),
        "wqT": bf16(np.asarray(Wq, np.float32).T),
        "wkvT": bf16(np.asarray(Wkv, np.float32).T),
        "woT": bf16(np.asarray(Wo, np.float32).T),
        "ind": ind,
        "indT": f32(ind.T),
    }
    return [dict(shared, x=np.ascontiguousarray(xr[i])) for i in range(B)]


def kernel(x, gamma, beta, Wq, bq, Wkv, bkv, Wo, bo):
    nc = _build()
    in_maps = _in_maps(x, gamma, beta, Wq, bq, Wkv, bkv, Wo, bo)
    res = bass_utils.run_bass_kernel_spmd(nc, in_maps, core_ids=list(range(NCORES)))
    out = np.stack([res.results[i]["out"] for i in range(B)], axis=0)
    return out.reshape(B, C, HW, HW).astype(np.float32)


# revision 16
# speedup vs baseline: 1.2892x; 1.2892x over previous
"""AttentionBlock (GroupNorm + single-head-group attention + out-proj + residual)
for Trainium2, data-parallel over batch across 8 NeuronCores.

Reference computation (per batch element, fp32 reference):
  hn  = GroupNorm32(x)                      # x: (C=512, L=1024)
  q   = Wq @ hn + bq ; kv = Wkv @ hn + bkv ; k, v = split(kv)
  per head h (8 heads, dh=64):
    dots = (q*s)^T (k*s), s = dh^-0.5       # scale applied to both q and k
    attn = softmax(dots, axis=s)
    out  = attn @ v^T  -> (dh, L)
  y = Wo @ out + bo + x

Layout strategy (avoids all large transposes):
  - channels on partitions for x/hn/q/k; v computed TRANSPOSED (L on
    partitions) directly from the projection (lhsT=hn);
  - scores computed transposed: dotsT[s,t] = kh^T qh (lhsT=kh, rhs=qh);
  - head pairs (2h, 2h+1) live at partition bases 0/64 of one chunk, so
    their K=64 dots matmuls run CONCURRENTLY in disjoint PE row groups
    via tile_position=(0,0)/(64,0);
  - softmax denominator via a ones-column appended to the vT weight tile
    (row 64 of the AV psum = sum_s exp); normalization applied at AV
    evacuation with a rank-1 (K=1) broadcast matmul;
  - exp() numerically safe without max-subtraction: |dots| < 0.5 here;
  - matmul operands in bf16 (fp32 matmul costs 2x on the PE and disables
    fast weight load); psum accumulation, groupnorm statistics, softmax
    normalization and the residual add stay fp32.
"""

import numpy as np
import ml_dtypes

import concourse.bass as bass
import concourse.mybir as mybir
import concourse.tile as tile
from concourse import bacc, bass_utils
from concourse.bass import ts

F32 = mybir.dt.float32
BF16 = mybir.dt.bfloat16
AF = mybir.ActivationFunctionType
OP = mybir.AluOpType

B = 8
C = 512
HW = 32
L = HW * HW  # 1024
H = 8
DH = C // H  # 64
G = 32
GS = C // G  # 16
EPS = 1e-5
SCALE2 = float(DH) ** -1.0  # (dh^-0.5) applied to BOTH q and k -> 1/dh on dots
P = 128
CCH = C // P  # 4 channel chunks
LCH = L // P  # 8 L chunks
NCORES = 8
VW = H * (DH + 1)  # 520: v^T tiles hold [64 v cols + 1 ones col] per head


def _body(tc, tensors):
    nc = tc.nc
    from contextlib import ExitStack

    ctx = ExitStack()
    with ctx:
        persist = ctx.enter_context(tc.tile_pool(name="persist", bufs=1))
        work = ctx.enter_context(tc.tile_pool(name="work", bufs=4))
        expp = ctx.enter_context(tc.tile_pool(name="expp", bufs=18))
        outp = ctx.enter_context(tc.tile_pool(name="outp", bufs=3))
        ps_wide = ctx.enter_context(tc.tile_pool(name="ps_wide", bufs=2, space="PSUM"))
        ps_av = ctx.enter_context(tc.tile_pool(name="ps_av", bufs=2, space="PSUM"))
        ps_bc = ctx.enter_context(tc.tile_pool(name="ps_bc", bufs=1, space="PSUM"))
        ps_small = ps_bc

        x_d = tensors["x"].ap()
        gamma_d = tensors["gamma"].ap()
        beta_d = tensors["beta"].ap()
        bq_d = tensors["bq"].ap()
        bkv_d = tensors["bkv"].ap()
        bo_d = tensors["bo"].ap()
        wqT_d = tensors["wqT"].ap()
        wkvT_d = tensors["wkvT"].ap()
        woT_d = tensors["woT"].ap()
        ind_d = tensors["ind"].ap()
        indT_d = tensors["indT"].ap()
        out_d = tensors["out"].ap()

        # ---------------- load inputs ----------------
        xt = []  # x, channel chunks (128, L)
        x3 = x_d.rearrange("(cc p) l -> cc p l", p=P)
        for cj in range(CCH):
            t = persist.tile([P, L], F32, tag=f"x{cj}")
            nc.sync.dma_start(t, x3[cj])
            xt.append(t)

        # per-channel params as (128, CCH) columns; column cj <-> channels cj*128..+128
        def load_cols(dram_vec, name, lo=0, n=C):
            t = persist.tile([P, CCH], F32, tag=name)
            nc.sync.dma_start(t, dram_vec[lo : lo + n].rearrange("(o p) -> p o", p=P))
            return t

        gamma_t = load_cols(gamma_d, "gamma")
        beta_t = load_cols(beta_d, "beta")
        bq_t = load_cols(bq_d, "bq")
        bk_t = load_cols(bkv_d, "bk", 0, C)
        bv_t = load_cols(bkv_d, "bv", C, C)
        bo_t = load_cols(bo_d, "bo")

        wq_t = persist.tile([P, CCH, C], BF16, tag="wq")
        nc.sync.dma_start(wq_t, wqT_d.rearrange("(cc p) o -> p cc o", p=P))
        wkv_t = persist.tile([P, CCH, 2 * C], BF16, tag="wkv")
        nc.sync.dma_start(wkv_t, wkvT_d.rearrange("(cc p) o -> p cc o", p=P))
        wo_t = persist.tile([P, CCH, C], BF16, tag="wo")
        nc.sync.dma_start(wo_t, woT_d.rearrange("(cc p) o -> p cc o", p=P))

        ones1 = persist.tile([1, DH], F32, tag="ones1")
        nc.vector.memset(ones1, 1.0)
        ind_t = persist.tile([P, CCH, G], F32, tag="ind")
        nc.sync.dma_start(ind_t, ind_d.rearrange("(cc p) g -> p cc g", p=P))
        indT_t = persist.tile([G, C], F32, tag="indT")
        nc.sync.dma_start(indT_t, indT_d)

        eps_t = persist.tile([G, 1], F32, tag="eps")
        nc.vector.memset(eps_t, EPS)

        # ---------------- GroupNorm ----------------
        # per-channel [sum, sumsq] -> group-reduce via indicator matmul
        stats = work.tile([P, CCH, 2], F32, tag="stats")
        for cj in range(CCH):
            sq = work.tile([P, L], F32, tag="sq")
            nc.scalar.activation(sq, xt[cj], AF.Square, accum_out=stats[:, cj, 1:2])
            nc.vector.reduce_sum(stats[:, cj, 0:1], xt[cj], axis=mybir.AxisListType.X)

        ps_stats = ps_small.tile([G, 2], F32, tag="ps")
        for cj in range(CCH):
            nc.tensor.matmul(
                ps_stats,
                ind_t[:, cj, :],
                stats[:, cj, :],
                start=(cj == 0),
                stop=(cj == CCH - 1),
            )

        # mv = [mean, rstd] per group (G partitions)
        mv = work.tile([G, 2], F32, tag="mv")
        inv_n = 1.0 / (GS * L)
        nc.scalar.mul(mv[:, 0:1], ps_stats[:, 0:1], inv_n)  # mean
        nc.scalar.mul(mv[:, 1:2], ps_stats[:, 1:2], inv_n)  # E[x^2]
        musq = work.tile([G, 1], F32, tag="musq")
        nc.vector.tensor_mul(musq, mv[:, 0:1], mv[:, 0:1])
        nc.vector.tensor_tensor(mv[:, 1:2], mv[:, 1:2], musq, OP.subtract)  # var
        nc.scalar.activation(mv[:, 1:2], mv[:, 1:2], AF.Sqrt, bias=eps_t)
        nc.vector.reciprocal(mv[:, 1:2], mv[:, 1:2])  # rstd

        # broadcast group stats back to channels: (G,2) -> (128,2) per chunk
        hn = []
        for cj in range(CCH):
            ps_bcst = ps_small.tile([P, 2], F32, tag="ps")
            nc.tensor.matmul(ps_bcst, indT_t[:, ts(cj, P)], mv, start=True, stop=True)
            mc = work.tile([P, 2], F32, tag="mc")
            nc.vector.tensor_copy(mc, ps_bcst)
            a = work.tile([P, 1], F32, tag="a_sc")
            b = work.tile([P, 1], F32, tag="b_sc")
            # a = rstd*gamma ; b = beta - mean*a
            nc.vector.tensor_mul(a, mc[:, 1:2], gamma_t[:, cj : cj + 1])
            nc.vector.tensor_mul(b, mc[:, 0:1], a)
            nc.vector.tensor_tensor(b, beta_t[:, cj : cj + 1], b, OP.subtract)
            t = persist.tile([P, L], BF16, tag=f"hn{cj}")
            nc.vector.tensor_scalar(
                t, xt[cj], scalar1=a, scalar2=b, op0=OP.mult, op1=OP.add
            )
            hn.append(t)

        # ---------------- projections ----------------
        # q (channels on partitions), pre-scaled by 1/dh; k (channels on partitions);
        # vT (L on partitions) with per-head ones-columns appended.
        q_t = [persist.tile([P, L], BF16, tag=f"q{oj}", name=f"q{oj}") for oj in range(CCH)]
        k_t = [persist.tile([P, L], BF16, tag=f"k{oj}", name=f"k{oj}") for oj in range(CCH)]
        for oj in range(CCH):
            for th in range(2):
                ps_q = ps_av.tile([P, 512], F32, tag="ps")
                for cj in range(CCH):
                    nc.tensor.matmul(
                        ps_q,
                        wq_t[:, cj, ts(oj, P)],
                        hn[cj][:, ts(th, 512)],
                        start=(cj == 0),
                        stop=(cj == CCH - 1),
                    )
                # q = (psum + bq) * (1/dh)
                nc.vector.tensor_scalar(
                    q_t[oj][:, ts(th, 512)],
                    ps_q,
                    scalar1=bq_t[:, oj : oj + 1],
                    scalar2=SCALE2,
                    op0=OP.add,
                    op1=OP.mult,
                )
                ps_k = ps_av.tile([P, 512], F32, tag="ps")
                for cj in range(CCH):
                    nc.tensor.matmul(
                        ps_k,
                        wkv_t[:, cj, ts(oj, P)],
                        hn[cj][:, ts(th, 512)],
                        start=(cj == 0),
                        stop=(cj == CCH - 1),
                    )
                nc.vector.tensor_scalar(
                    k_t[oj][:, ts(th, 512)],
                    ps_k,
                    scalar1=bk_t[:, oj : oj + 1],
                    scalar2=None,
                    op0=OP.add,
                )

        # vT: out[l, i] = sum_c hn[c, l] * Wv^T[c, i]  (lhsT = hn chunks)
        vT = [persist.tile([P, VW], BF16, tag=f"vT{lj}", name=f"vT{lj}") for lj in range(LCH)]
        for lj in range(LCH):
            v3 = vT[lj].rearrange("p (h w) -> p h w", w=DH + 1)
            nc.vector.memset(v3[:, :, DH : DH + 1], 1.0)
            ps_v = ps_av.tile([P, 512], F32, tag="ps")
            for cj in range(CCH):
                nc.tensor.matmul(
                    ps_v,
                    hn[cj][:, ts(lj, P)],
                    wkv_t[:, cj, C : 2 * C],
                    start=(cj == 0),
                    stop=(cj == CCH - 1),
                )
            # v bias is folded into the attention output (rows sum to 1).
            # single strided copy: psum (p,(h d)) -> vT (p,h,0:DH)
            nc.vector.tensor_copy(
                v3[:, :, 0:DH], ps_v.rearrange("p (h d) -> p h d", d=DH)
            )

        # ---------------- attention, head-pair pipelined ----------------
        av_t = [persist.tile([P, L], BF16, tag=f"av{oj}", name=f"av{oj}") for oj in range(CCH)]
        exp_tiles: dict = {}

        def emit_dots_exp_pair(hp):
            oj = hp
            qA, qB = q_t[oj][0:DH, :], q_t[oj][DH:P, :]
            kA, kB = k_t[oj][0:DH, :], k_t[oj][DH:P, :]
            for sj in range(LCH):
                psA = ps_wide.tile([P, L], F32, tag="ps")
                psB = ps_wide.tile([P, L], F32, tag="ps")
                for th in range(2):
                    nc.tensor.matmul(
                        psA[:, ts(th, 512)],
                        kA[:, ts(sj, P)],
                        qA[:, ts(th, 512)],
                        start=True,
                        stop=True,
                        tile_position=(0, 0),
                    )
                    nc.tensor.matmul(
                        psB[:, ts(th, 512)],
                        kB[:, ts(sj, P)],
                        qB[:, ts(th, 512)],
                        start=True,
                        stop=True,
                        tile_position=(64, 0),
                    )
                eA = expp.tile([P, L], BF16, tag="exp")
                nc.scalar.activation(eA, psA, AF.Exp)
                eB = expp.tile([P, L], BF16, tag="exp")
                nc.scalar.activation(eB, psB, AF.Exp)
                exp_tiles[(2 * hp, sj)] = eA
                exp_tiles[(2 * hp + 1, sj)] = eB

        def emit_av(h):
            oj, base = h // 2, DH * (h % 2)
            w0 = h * (DH + 1)
            for th in range(2):
                ps_o = ps_av.tile([DH + 1, 512], F32, tag="ps")
                for sj in range(LCH):
                    nc.tensor.matmul(
                        ps_o,
                        vT[sj][:, w0 : w0 + DH + 1],
                        exp_tiles[(h, sj)][:, ts(th, 512)],
                        start=(sj == 0),
                        stop=(sj == LCH - 1),
                    )
                # 1/sumexp: cross-partition copy of the psum row to p0
                # (plain DVE ops may cross partitions; the custom fast-recip
                # op must run same-partition from SBUF), fast recip, then
                # rank-1 PE broadcast and multiply.
                rec_raw = work.tile([1, 512], F32, tag="rec_raw")
                nc.vector.tensor_copy(rec_raw, ps_o[DH : DH + 1, :])
                rec = work.tile([1, 512], F32, tag="rec")
                nc.vector.reciprocal_approx_fast(rec, rec_raw)
                ps_b = ps_bc.tile([DH, 512], F32, tag="psb")
                nc.tensor.matmul(ps_b, ones1, rec, start=True, stop=True)
                bc = work.tile([DH, 512], F32, tag="bc")
                nc.vector.tensor_copy(bc, ps_b)
                nc.vector.tensor_tensor(
                    av_t[oj][base : base + DH, ts(th, 512)],
                    ps_o[:DH, :],
                    bc,
                    OP.mult,
                )
            for sj in range(LCH):
                del exp_tiles[(h, sj)]

        emit_dots_exp_pair(0)
        for hp in range(1, CCH):
            emit_dots_exp_pair(hp)
            emit_av(2 * (hp - 1))
            emit_av(2 * (hp - 1) + 1)
        emit_av(H - 2)
        emit_av(H - 1)

        # v bias: out[d,t] += bv[d]  (rows of av are inner channels)
        # (softmax rows sum to one, so +bv on v adds bv to every column)
        for oj in range(CCH):
            nc.vector.tensor_scalar(
                av_t[oj], av_t[oj], scalar1=bv_t[:, oj : oj + 1], scalar2=None, op0=OP.add
            )

        # ---------------- output projection + residual ----------------
        out3 = out_d.rearrange("(cc p) l -> cc p l", p=P)
        for oj in range(CCH):
            ot = outp.tile([P, L], F32, tag="ot")
            for th in range(2):
                ps_f = ps_av.tile([P, 512], F32, tag="ps")
                for cj in range(CCH):
                    nc.tensor.matmul(
                        ps_f,
                        wo_t[:, cj, ts(oj, P)],
                        av_t[cj][:, ts(th, 512)],
                        start=(cj == 0),
                        stop=(cj == CCH - 1),
                    )
                nc.vector.tensor_scalar(
                    ot[:, ts(th, 512)],
                    ps_f,
                    scalar1=bo_t[:, oj : oj + 1],
                    scalar2=None,
                    op0=OP.add,
                )
            nc.vector.tensor_add(ot, ot, xt[oj])
            nc.sync.dma_start(out3[oj], ot)


_CACHE = {}


def _build():
    if "nc" in _CACHE:
        return _CACHE["nc"]
    nc = bacc.Bacc("TRN2", target_bir_lowering=False, debug=False, num_devices=NCORES)
    tensors = {}
    specs = [
        ("x", (C, L), F32),
        ("gamma", (C,), F32),
        ("beta", (C,), F32),
        ("bq", (C,), F32),
        ("bkv", (2 * C,), F32),
        ("bo", (C,), F32),
        ("wqT", (C, C), BF16),
        ("wkvT", (C, 2 * C), BF16),
        ("woT", (C, C), BF16),
        ("ind", (C, G), F32),
        ("indT", (G, C), F32),
    ]
    for name, shape, dt in specs:
        tensors[name] = nc.dram_tensor(name, shape, dt, kind="ExternalInput")
    tensors["out"] = nc.dram_tensor("out", (C, L), F32, kind="ExternalOutput")
    with tile.TileContext(nc) as tc:
        _body(tc, tensors)
    nc.compile()
    _CACHE["nc"] = nc
    return nc


def _in_maps(x, gamma, beta, Wq, bq, Wkv, bkv, Wo, bo):
    f32 = lambda a: np.ascontiguousarray(np.asarray(a, dtype=np.float32))
    bf16 = lambda a: np.ascontiguousarray(
        np.asarray(a, dtype=np.float32).astype(ml_dtypes.bfloat16)
    )
    xr = f32(x).reshape(B, C, L)
    ind = np.zeros((C, G), np.float32)
    ind[np.arange(C), np.arange(C) // GS] = 1.0
    shared = {
        "gamma": f32(gamma),
        "beta": f32(beta),
        "bq": f32(bq),
        "bkv": f32(bkv),
        "bo": f32(bo),
        "wqT": bf16(np.asarray(Wq, np.float32).T),
        "wkvT": bf16(np.asarray(Wkv, np.float32).T),
        "woT": bf16(np.asarray(Wo, np.float32).T),
        "ind": ind,
        "indT": f32(ind.T),
    }
    return [dict(shared, x=np.ascontiguousarray(xr[i])) for i in range(B)]


def kernel(x, gamma, beta, Wq, bq, Wkv, bkv, Wo, bo):
    nc = _build()
    in_maps = _in_maps(x, gamma, beta, Wq, bq, Wkv, bkv, Wo, bo)
    res = bass_utils.run_bass_kernel_spmd(nc, in_maps, core_ids=list(range(NCORES)))
    out = np.stack([res.results[i]["out"] for i in range(B)], axis=0)
    return out.reshape(B, C, HW, HW).astype(np.float32)
